# revision 1
# baseline (speedup 1.0000x reference)
"""MCorr1d Trainium2 kernel (8 NeuronCores).

Problem (hardcoded from spec):
  in_    [1024, 64, 512]  fp32   (X, N, C_in)
  weight [16, 512, 512]   fp32   (KW, C_in, C_out)
  bias   [512]            fp32
  out    [64, 64, 512]    fp32   (Y, N, C_out)

  out[y, n, o] = bias[o] + sum_{w=0}^{15} sum_c in_[(y+1)*(w+1)-1, n, c] * weight[w, c, o]

Sharding: data-parallel over N (batch): core i handles n in [8*i, 8*i+8).
Each core computes rows r = y*8 + n_local (512 rows) of out[., n_slice, .]
as 16 accumulating GEMMs of [512,512] @ [512,512].

Host packs, per core, A_pack[w, c, r] = in_[(y+1)*(w+1)-1, n0+n, c]
(im2col-style gather+transpose) so every DMA is contiguous and the
tensor engine consumes tiles directly with no on-device transposes.

Precision modes:
  fp32   : plain float32 matmuls (4 cycles/row on PE), rel err ~1e-6
  fp32r  : float32r single-pass matmuls (1 cycle/row), rel err ~1.6e-4
  bf16x3 : hi/lo bf16 split, 3 matmuls (hi@hi + hi@lo + lo@hi), rel err ~5e-6
  bf16   : plain bf16 (half DMA bytes), rel err ~2e-3
  bf16r  : bf16, W|A packed per tap into one DRAM tensor (single 1MB DMA
           per tap, early taps k-chunked for fast pipeline fill), all 16
           tap tiles SBUF-resident, dummy warm-up matmuls during the DMA
           fill to absorb the PE p-state/HAM cold ramp, staggered
           accumulation-group completion so drains overlap compute, bias
           folded into the PSUM->SBUF drain on DVE, rel err ~2e-3

Measured (loop-diff, per-iteration device time on 8 trn2 cores):
  bf16x3 baseline ~199us -> bf16 ~92us -> bf16r 66.5us (best; ~73-76us
  after hours of sustained benching progressively throttled the shared
  device). TimelineSim body latency: 63.1us. Compute roofline is
  256 bf16 matmuls x 213ns = 54.5us/core; the ~8.6us residual is DMA
  first-byte latency, the final drain chain, and the kernel epilogue
  barrier -- all handoff constants, with every reorderable stage
  already overlapped.
"""

import contextlib

import numpy as np

X_LEN, N_BATCH, C_IN = 1024, 64, 512
KW, C_OUT = 16, 512
Y_OUT = 64
N_CORES = 8
N_PER = N_BATCH // N_CORES  # 8
ROWS = Y_OUT * N_PER  # 512
KC = C_IN // 128  # 4 k-chunks
MC = ROWS // 128  # 4 m-chunks
H_LEN = (256, 256)  # o-split of the last row group's accumulation
H_OFF = (0, 256)

MODE = "bf16r"
WARM_MMS = 5  # dummy N=512 matmuls issued during the DMA fill to warm the PE

_XS = np.array([[(y + 1) * (w + 1) - 1 for y in range(Y_OUT)] for w in range(KW)])


def _build_nc(mode):
    return _build_nc_reps(mode, 1)


def _build_nc_reps(mode, reps, loop_n=0):
    import concourse.mybir as mybir
    import concourse.tile as tile
    from concourse import bacc

    f32 = mybir.dt.float32
    if mode == "fp32":
        mdt = f32
    elif mode == "fp32r":
        mdt = mybir.dt.float32r
    elif mode in ("bf16x3", "bf16", "bf16r"):
        mdt = mybir.dt.bfloat16
    else:
        raise ValueError(mode)

    nc = bacc.Bacc("TRN2", target_bir_lowering=False, debug=False,
                   num_devices=N_CORES)

    # Per-core DRAM tensors (SPMD: same program, different data per core).
    ins = {}
    if mode == "bf16r":
        names = ("aw",)
    elif mode == "bf16x3":
        names = ("a_hi", "w_hi", "w_lo", "a_lo")
    else:
        names = ("a", "w")
    for nm in names:
        if nm == "aw":
            shp = [KW, C_IN, ROWS + C_OUT]
        elif nm.startswith("a"):
            shp = [KW, C_IN, ROWS]
        else:
            shp = [KW, C_IN, C_OUT]
        ins[nm] = nc.dram_tensor(nm, shp, mdt, kind="ExternalInput").ap()
    if mode == "bf16r":
        bias_t = nc.dram_tensor("bias", [128, C_OUT], f32,
                                kind="ExternalInput").ap()
        ones_t = None
    else:
        bias_t = nc.dram_tensor("bias", [1, C_OUT], mybir.dt.float32r,
                                kind="ExternalInput").ap()
        ones_t = nc.dram_tensor("ones", [1, 128], mybir.dt.float32r,
                                kind="ExternalInput").ap()
    out_t = nc.dram_tensor("out", [ROWS, C_OUT], f32, kind="ExternalOutput").ap()

    with tile.TileContext(nc) as tc:
        nbufs = KW if mode == "bf16r" else 4
        with tc.tile_pool(name="asb", bufs=nbufs) as asb, \
             tc.tile_pool(name="csb", bufs=1) as csb, \
             tc.tile_pool(name="osb", bufs=2) as osb, \
             tc.tile_pool(name="ps", bufs=1, space="PSUM") as ps:

            # Constants (outside the timing loop body)
            if mode == "bf16r":
                bias_sb = csb.tile([128, C_OUT], f32, tag="bias")
                warm_sb = csb.tile([128, C_OUT], mdt, tag="warm")
                nc.vector.memset(warm_sb[:], 0.0)
                ones_sb = None
            else:
                bias_sb = csb.tile([1, C_OUT], mybir.dt.float32r, tag="bias")
                nc.sync.dma_start(bias_sb[:], bias_t[:])
                ones_sb = csb.tile([1, 128], mybir.dt.float32r, tag="ones")
                nc.sync.dma_start(ones_sb[:], ones_t[:])

            loop_cm = (tc.For_i(0, loop_n, 1) if loop_n
                       else contextlib.nullcontext())
            with loop_cm:
                for _rep in range(reps):
                    if mode == "bf16r":
                        _emit_body_r(nc, mdt, f32, asb, osb, ps,
                                     bias_sb, warm_sb, bias_t, ins, out_t)
                    else:
                        _emit_body(nc, mode, mdt, f32, asb, osb, ps,
                                   bias_sb, ones_sb, ins, out_t)

    nc.compile()
    return nc


def _emit_body(nc, mode, mdt, f32, asb, osb, ps, bias_sb, ones_sb, ins, out_t):
    # Output accumulators: 4 PSUM banks of [128, 512]
    acc = [ps.tile([128, C_OUT], f32, name=f"acc{m}", tag=f"acc{m}")
           for m in range(MC)]

    # Bias as rank-1 matmul opens each accumulation group.
    for m in range(MC):
        nc.tensor.matmul(acc[m][:], ones_sb[:], bias_sb[:],
                         start=True, stop=False)

    for w in range(KW):
        tiles = {}
        for nm, ap in ins.items():
            fd = ROWS if nm.startswith("a") else C_OUT
            t = asb.tile([128, KC, fd], mdt, name=nm + "_t", tag=nm)
            nc.sync.dma_start(t[:], ap[w].rearrange("(k p) f -> p k f", p=128))
            tiles[nm] = t
        if mode == "bf16x3":
            pairs = [(tiles["a_hi"], tiles["w_hi"]),
                     (tiles["a_hi"], tiles["w_lo"]),
                     (tiles["a_lo"], tiles["w_hi"])]
        else:
            pairs = [(tiles["a"], tiles["w"])]
        last_w = (w == KW - 1)
        for m in range(MC):
            for k in range(KC):
                for pi, (at, wt) in enumerate(pairs):
                    stop = (last_w and k == KC - 1 and pi == len(pairs) - 1)
                    nc.tensor.matmul(
                        acc[m][:],
                        at[:, k, m * 128:(m + 1) * 128],
                        wt[:, k, :],
                        start=False, stop=stop)

    for m in range(MC):
        o_sb = osb.tile([128, C_OUT], f32, tag="o")
        nc.vector.tensor_copy(o_sb[:], acc[m][:])
        nc.sync.dma_start(out_t[m * 128:(m + 1) * 128, :], o_sb[:])


def _emit_body_r(nc, mdt, f32, asb, osb, ps, bias_sb, warm_sb, bias_t,
                 ins, out_t):
    import concourse.mybir as mybir

    aw = ins["aw"]
    # m0..m2 accumulate in full [128, 512] banks; m3 (the group whose
    # drain ends the kernel) accumulates in two half-width banks so the
    # first half can stop, drain, and start its output DMA while the
    # second half's final matmuls still run.
    acc = [ps.tile([128, C_OUT], f32, name=f"acc{m}", tag=f"acc{m}")
           for m in range(MC - 1)]
    acc3 = [ps.tile([128, H_LEN[h]], f32, name=f"acc3{h}", tag=f"acc3{h}")
            for h in range(2)]

    # Dummy matmuls on memset data keep the PE busy through the DMA fill
    # so the p-state/HAM ramp is paid before real data arrives.
    if WARM_MMS:
        wps = ps.tile([128, C_OUT], f32, name="warm_ps", tag="warm_ps")
        for _ in range(WARM_MMS):
            nc.tensor.matmul(wps[:], warm_sb[:, :128], warm_sb[:],
                             start=True, stop=True)

    # All 16 tap tiles are resident; one 1MB DMA per tap. Layout per tap
    # is [W (512 cols) | A (512 cols)] so the very first matmul needs only
    # a 160KB prefix of tap 0. Early taps are k-chunked so the PE is never
    # paced by a whole-tile DMA completion.
    tiles = []
    for w in range(KW):
        t = asb.tile([128, KC, C_OUT + ROWS], mdt, name=f"aw{w}", tag="aw")
        src = aw[w].rearrange("(k p) f -> p k f", p=128)
        if w == 0:
            nc.sync.dma_start(t[:, 0, :C_OUT + 128], src[:, 0, :C_OUT + 128])
            nc.sync.dma_start(t[:, 0, C_OUT + 128:], src[:, 0, C_OUT + 128:])
            for k in range(1, KC):
                nc.sync.dma_start(t[:, k, :], src[:, k, :])
        elif w <= 3:
            for k in range(KC):
                nc.sync.dma_start(t[:, k, :], src[:, k, :])
        else:
            nc.sync.dma_start(t[:], src)
        tiles.append(t)
    # Bias lands after the tap stream; it is only read by the drains.
    nc.sync.dma_start(bias_sb[:], bias_t[:])

    def mm(w, m, k):
        lhsT = tiles[w][:, k, C_OUT + m * 128:C_OUT + (m + 1) * 128]
        first, last = (w == 0 and k == 0), (w == KW - 1 and k == KC - 1)
        if m < MC - 1:
            nc.tensor.matmul(acc[m][:], lhsT, tiles[w][:, k, :C_OUT],
                             start=first, stop=last)
        else:
            for h in range(2):
                nc.tensor.matmul(
                    acc3[h][:], lhsT,
                    tiles[w][:, k, H_OFF[h]:H_OFF[h] + H_LEN[h]],
                    start=first, stop=last)

    def mm3_half(w, h, k):
        nc.tensor.matmul(
            acc3[h][:],
            tiles[w][:, k, C_OUT + (MC - 1) * 128:C_OUT + MC * 128],
            tiles[w][:, k, H_OFF[h]:H_OFF[h] + H_LEN[h]],
            start=False, stop=(w == KW - 1 and k == KC - 1))

    # Phase A: taps 0..SPLIT_W-1 for all row groups (DMA-paced region).
    # Phase B: remaining taps per group, so groups complete staggered and
    # the drains of m0..m2 hide under m1..m3's remaining matmuls. The last
    # tap of m3 runs half 0's k-chunks before half 1's, staggering the two
    # stop events by ~0.4us.
    split_w = KW - 3
    for w in range(split_w):
        for m in range(MC):
            for k in range(KC):
                mm(w, m, k)
    for m in range(MC - 1):
        for w in range(split_w, KW):
            for k in range(KC):
                mm(w, m, k)
    for w in range(split_w, KW - 1):
        for k in range(KC):
            mm(w, MC - 1, k)
    for h in range(2):
        for k in range(KC):
            mm3_half(KW - 1, h, k)

    for m in range(MC - 1):
        o_sb = osb.tile([128, C_OUT], f32, tag="o")
        nc.vector.tensor_tensor(o_sb[:], acc[m][:], bias_sb[:],
                                mybir.AluOpType.add)
        nc.sync.dma_start(out_t[m * 128:(m + 1) * 128, :], o_sb[:])
    # m3: each half drains as soon as its own accumulation stops.
    m = MC - 1
    o_sb = osb.tile([128, C_OUT], f32, tag="o")
    for h in range(2):
        sl = slice(H_OFF[h], H_OFF[h] + H_LEN[h])
        nc.vector.tensor_tensor(o_sb[:, sl], acc3[h][:],
                                bias_sb[:, sl], mybir.AluOpType.add)
        nc.sync.dma_start(out_t[m * 128:(m + 1) * 128, sl], o_sb[:, sl])


_NC_CACHE = {}


def _get_nc(mode):
    if mode not in _NC_CACHE:
        _NC_CACHE[mode] = _build_nc(mode)
    return _NC_CACHE[mode]


def _pack_inputs(in_, weight, bias, mode):
    """Host-side gather/transpose pack. Returns list of per-core input maps."""
    import ml_dtypes

    in_ = np.asarray(in_, dtype=np.float32)
    weight = np.asarray(weight, dtype=np.float32)
    bias = np.asarray(bias, dtype=np.float32)

    # G[w, y, n, c] = in_[(y+1)(w+1)-1, n, c]
    G = in_[_XS.reshape(-1)].reshape(KW, Y_OUT, N_BATCH, C_IN)
    # A_all[w, c, y, n]
    A_all = np.ascontiguousarray(G.transpose(0, 3, 1, 2))

    ones = np.ones((1, 128), np.float32)
    bias2 = bias.reshape(1, C_OUT)

    def split(x):
        hi = x.astype(ml_dtypes.bfloat16)
        lo = (x - hi.astype(np.float32)).astype(ml_dtypes.bfloat16)
        return hi, lo

    if mode == "bf16x3":
        w_hi, w_lo = split(weight)
    elif mode in ("bf16", "bf16r"):
        w_b = weight.astype(ml_dtypes.bfloat16)

    in_maps = []
    for c in range(N_CORES):
        n0 = c * N_PER
        a_c = np.ascontiguousarray(
            A_all[:, :, :, n0:n0 + N_PER]).reshape(KW, C_IN, ROWS)
        if mode == "bf16r":
            awt = np.concatenate(
                [np.broadcast_to(w_b, (KW, C_IN, C_OUT)),
                 a_c.astype(ml_dtypes.bfloat16)], axis=2)
            m = {"aw": np.ascontiguousarray(awt),
                 "bias": np.tile(bias2, (128, 1))}
        else:
            m = {"bias": bias2, "ones": ones}
            if mode == "bf16x3":
                a_hi, a_lo = split(a_c)
                m.update(a_hi=a_hi, a_lo=a_lo, w_hi=w_hi, w_lo=w_lo)
            elif mode == "bf16":
                m.update(a=a_c.astype(ml_dtypes.bfloat16), w=w_b)
            else:
                m.update(a=a_c, w=weight)
        in_maps.append(m)
    return in_maps


def kernel(in_, weight, bias):
    from concourse.bass_utils import run_bass_kernel_spmd

    nc = _get_nc(MODE)
    in_maps = _pack_inputs(in_, weight, bias, MODE)
    res = run_bass_kernel_spmd(nc, in_maps, core_ids=list(range(N_CORES)))
    # Each core returns out [ROWS, C_OUT] with rows = y*N_PER + n_local.
    parts = [r["out"].reshape(Y_OUT, N_PER, C_OUT) for r in res.results]
    return np.concatenate(parts, axis=1).astype(np.float32)

